# Initial kernel scaffold
#
"""Two-layer GCN (DGL GraphConv norm='both') on 8 Trainium2 NeuronCores.

Strategy (graph/data parallel, per sharding hint):
  - Nodes are range-partitioned across the 8 cores (1250 each); each core
    owns the dst-side segment_sum for its node range.
  - Host pre-sorts each core's incoming edges by dst, groups them into
    128-row dst windows, and pads each window's edge list to a uniform
    block count so all 8 cores share one SPMD program.
  - Layer-1 aggregation: dma_gather pulls (norm_src-scaled) source feature
    rows (bf16) from HBM; a per-block one-hot matrix M (built on-chip with
    iota + is_equal from precomputed local-dst ids) turns the segment sum
    into TensorEngine matmuls accumulating in PSUM:
        agg[dst,:] += M[edge,dst]^T @ Xg[edge,:]
  - H1 = (agg @ W1) * norm_dst + b1 (norm commutes through W1), ReLU, then
    z = (relu * norm_src) @ W2 is computed locally and AllGathered (bf16,
    padded to 128 cols so gather rows are 256B-aligned).
  - Layer-2 aggregation: same gather + one-hot matmul scheme over z,
    then out = agg2 * norm_dst + b2.
"""

import numpy as np
import ml_dtypes

BF16 = ml_dtypes.bfloat16
N_CORES = 8

LAST_STATS = {}


def _prep(features, W1, b1, W2, b2, src, dst):
    N, F = features.shape
    H = W1.shape[1]
    C = W2.shape[1]
    E = src.shape[0]
    assert N % N_CORES == 0
    npc = N // N_CORES            # nodes per core
    NT = (npc + 127) // 128       # dst windows per core
    npc_pad = NT * 128

    src = np.asarray(src, np.int64)
    dst = np.asarray(dst, np.int64)

    out_deg = np.bincount(src, minlength=N).astype(np.float32)
    in_deg = np.bincount(dst, minlength=N).astype(np.float32)
    norm_src = (1.0 / np.sqrt(np.clip(out_deg, 1.0, None))).astype(np.float32)
    norm_dst = (1.0 / np.sqrt(np.clip(in_deg, 1.0, None))).astype(np.float32)

    # norm_src folded into the gathered feature table (host-side sharding prep)
    featb = (np.asarray(features, np.float32) * norm_src[:, None]).astype(BF16)

    order = np.argsort(dst, kind="stable")
    ds = dst[order]
    ss = src[order]

    # per (core, window) counts
    i0 = np.empty((N_CORES, NT), np.int64)
    i1 = np.empty((N_CORES, NT), np.int64)
    for c in range(N_CORES):
        for w in range(NT):
            lo = c * npc + w * 128
            hi = min(lo + 128, (c + 1) * npc)
            i0[c, w] = np.searchsorted(ds, lo, "left")
            i1[c, w] = np.searchsorted(ds, hi, "left")
    counts = i1 - i0
    B = int(np.ceil(counts.max() / 128))      # blocks per window (uniform)
    EW = B * 128                              # padded edges per window
    NE = NT * EW                              # padded edges per core
    NBLK = NT * B

    idx1 = np.zeros((N_CORES, 128, NE // 16), np.int16)
    idx2 = np.zeros((N_CORES, 128, NE // 16), np.int16)
    dlw = np.zeros((N_CORES, 128, NBLK), BF16)
    ndst_t = np.zeros((N_CORES, 128, NT), np.float32)
    nso_t = np.zeros((N_CORES, 128, NT), np.float32)

    for c in range(N_CORES):
        s1 = np.zeros(NE, np.int64)
        dl = np.full(NE, -1.0, np.float32)
        for w in range(NT):
            a, b = i0[c, w], i1[c, w]
            cnt = b - a
            pos = w * EW
            s1[pos:pos + cnt] = ss[a:b]
            dl[pos:pos + cnt] = (ds[a:b] - (c * npc + w * 128)).astype(np.float32)
        s2 = (s1 // npc) * npc_pad + (s1 % npc)   # row in AllGathered z
        idx1[c] = np.tile(s1.reshape(NE // 16, 16).T.astype(np.int16), (8, 1))
        idx2[c] = np.tile(s2.reshape(NE // 16, 16).T.astype(np.int16), (8, 1))
        dlw[c] = dl.reshape(NBLK, 128).T.astype(BF16)

        own_nd = np.ones(npc_pad, np.float32)
        own_nd[:npc] = norm_dst[c * npc:(c + 1) * npc]
        own_ns = np.ones(npc_pad, np.float32)
        own_ns[:npc] = norm_src[c * npc:(c + 1) * npc]
        ndst_t[c] = own_nd.reshape(NT, 128).T
        nso_t[c] = own_ns.reshape(NT, 128).T

    shapes = dict(N=N, F=F, H=H, C=C, NT=NT, B=B, NE=NE, npc=npc)
    in_maps = []
    W1b = np.asarray(W1, np.float32).astype(BF16)
    W2b = np.asarray(W2, np.float32).astype(BF16)
    b1f = np.asarray(b1, np.float32)[None, :]
    b2f = np.asarray(b2, np.float32)[None, :]
    for c in range(N_CORES):
        in_maps.append(dict(
            featb=featb, idx1=idx1[c], idx2=idx2[c], dlbf=dlw[c],
            ndst=ndst_t[c], nso=nso_t[c],
            w1=W1b, w2=W2b, b1=b1f, b2=b2f,
        ))
    return shapes, in_maps


def _build(N, F, H, C, NT, B, NE, variant="full", num_devices=N_CORES):
    import concourse.bacc as bacc
    import concourse.mybir as mybir
    from concourse import tile

    dt = mybir.dt
    alu = mybir.AluOpType
    KF = F // 128   # feat chunks
    KH = H // 128   # hidden chunks
    EW = B * 128
    ZROWS = N_CORES * NT * 128

    nc = bacc.Bacc("TRN2", target_bir_lowering=False, debug=False,
                   num_devices=num_devices, num_swdge_queues=4)

    featb_d = nc.dram_tensor("featb", [N, F], dt.bfloat16, kind="ExternalInput")
    idx1_d = nc.dram_tensor("idx1", [128, NE // 16], dt.int16, kind="ExternalInput")
    idx2_d = nc.dram_tensor("idx2", [128, NE // 16], dt.int16, kind="ExternalInput")
    dlbf_d = nc.dram_tensor("dlbf", [128, NT * B], dt.bfloat16, kind="ExternalInput")
    ndst_d = nc.dram_tensor("ndst", [128, NT], dt.float32, kind="ExternalInput")
    nso_d = nc.dram_tensor("nso", [128, NT], dt.float32, kind="ExternalInput")
    w1_d = nc.dram_tensor("w1", [F, H], dt.bfloat16, kind="ExternalInput")
    w2_d = nc.dram_tensor("w2", [H, C], dt.bfloat16, kind="ExternalInput")
    b1_d = nc.dram_tensor("b1", [1, H], dt.float32, kind="ExternalInput")
    b2_d = nc.dram_tensor("b2", [1, C], dt.float32, kind="ExternalInput")
    out_d = nc.dram_tensor("out", [NT * 128, C], dt.float32, kind="ExternalOutput")

    with tile.TileContext(nc) as tc:
        with (
            tc.tile_pool(name="const", bufs=1) as const,
            tc.tile_pool(name="dram", bufs=1, space="DRAM") as dram,
            tc.tile_pool(name="xg", bufs=3) as xg_pool,
            tc.tile_pool(name="mp", bufs=3) as m_pool,
            tc.tile_pool(name="work", bufs=3) as work,
            tc.tile_pool(name="ps_agg", bufs=2, space="PSUM") as ps_agg,
            tc.tile_pool(name="ps_tr", bufs=2, space="PSUM") as ps_tr,
            tc.tile_pool(name="ps_h", bufs=2, space="PSUM") as ps_h,
        ):
            # ---- constants ----
            idx1_t = const.tile([128, NE // 16], dt.int16)
            nc.sync.dma_start(idx1_t[:], idx1_d.ap())
            idx2_t = const.tile([128, NE // 16], dt.int16)
            nc.sync.dma_start(idx2_t[:], idx2_d.ap())
            dlbf_t = const.tile([128, NT * B], dt.bfloat16)
            nc.sync.dma_start(dlbf_t[:], dlbf_d.ap())
            ndst_t = const.tile([128, NT], dt.float32)
            nc.sync.dma_start(ndst_t[:], ndst_d.ap())
            nso_t = const.tile([128, NT], dt.float32)
            nc.sync.dma_start(nso_t[:], nso_d.ap())

            w1_sb = const.tile([128, KF, H], dt.bfloat16)
            nc.sync.dma_start(w1_sb[:], w1_d.ap().rearrange("(k p) n -> p k n", p=128))
            w2_sb = const.tile([128, KH, C], dt.bfloat16)
            nc.sync.dma_start(w2_sb[:], w2_d.ap().rearrange("(k p) n -> p k n", p=128))
            b1_sb = const.tile([1, H], dt.float32)
            nc.sync.dma_start(b1_sb[:], b1_d.ap())
            b2_sb = const.tile([1, C], dt.float32)
            nc.sync.dma_start(b2_sb[:], b2_d.ap())

            iota_bf = const.tile([128, B, 128], dt.bfloat16)
            nc.gpsimd.iota(iota_bf[:], pattern=[[0, B], [1, 128]], base=0,
                           channel_multiplier=0,
                           allow_small_or_imprecise_dtypes=True)
            iota_col = const.tile([128, 1], dt.float32)
            nc.gpsimd.iota(iota_col[:], pattern=[[0, 1]], base=0,
                           channel_multiplier=1,
                           allow_small_or_imprecise_dtypes=True)
            ident_bf = const.tile([128, 128], dt.bfloat16)
            nc.vector.tensor_scalar(ident_bf[:], iota_bf[:, 0, :], iota_col[:],
                                    None, alu.is_equal)

            # bias rows broadcast across partitions via ones-column matmul
            ones_sb = const.tile([1, 128], dt.float32)
            nc.vector.memset(ones_sb[:], 1.0)
            b1_ps = ps_h.tile([128, H], dt.float32, tag="h")
            nc.tensor.matmul(b1_ps[:], lhsT=ones_sb[:], rhs=b1_sb[:],
                             start=True, stop=True)
            b1_bc = const.tile([128, H], dt.float32)
            nc.vector.tensor_copy(b1_bc[:], b1_ps[:])
            b2_ps = ps_h.tile([128, C], dt.float32, tag="zn")
            nc.tensor.matmul(b2_ps[:], lhsT=ones_sb[:], rhs=b2_sb[:],
                             start=True, stop=True)
            b2_bc = const.tile([128, C], dt.float32)
            nc.vector.tensor_copy(b2_bc[:], b2_ps[:])

            # z rows padded to 256 bf16 cols = 512B: dma_gather rows below
            # 512B crash the Q7 ucode; gathered cols C.. are never consumed.
            cc_in = dram.tile([NT * 128, 256], dt.bfloat16)
            z_full = dram.tile([ZROWS, 256], dt.bfloat16, addr_space="Shared")

            # ---- layer 1 ----
            # dma_gather is limited to 1024 idxs per instruction (64 descs per
            # SDMA lane = the single_packet ceiling); larger crashes the Q7.
            GC = 8  # blocks (of 128 idxs) per gather instruction

            for w in range(NT):
                xg = xg_pool.tile([128, B, F], dt.bfloat16, tag="xg")
                if variant == "no_gather":
                    nc.vector.memset(xg[:], 0.25)
                elif variant == "no_l1gather":
                    nc.vector.memset(xg[:, 0, 0:16], 0.25)
                else:
                    for g in range(0, B, GC):
                        nb = min(GC, B - g)
                        c0 = (w * B + g) * 8
                        nc.gpsimd.dma_gather(
                            xg[:, g:g + nb, :], featb_d.ap(),
                            idx1_t[:, c0:c0 + nb * 8],
                            nb * 128, nb * 128, F,
                            queue_num=(w * ((B + GC - 1) // GC) + g // GC) % 4)
                m1 = m_pool.tile([128, B, 128], dt.bfloat16, tag="m")
                nc.vector.tensor_tensor(
                    m1[:], iota_bf[:],
                    dlbf_t[:, w * B:(w + 1) * B].broadcast_to((128, B, 128)),
                    alu.is_equal)
                agg = ps_agg.tile([128, F], dt.float32, tag="agg")
                NMM = 32 if variant == "tiny_mm" else F
                for b in range(B):
                    nc.tensor.matmul(agg[:, 0:NMM], lhsT=m1[:, b, :],
                                     rhs=xg[:, b, 0:NMM],
                                     start=(b == 0), stop=(b == B - 1))
                aggc = work.tile([128, F], dt.bfloat16, tag="aggc")
                nc.vector.tensor_copy(aggc[:], agg[:])
                if variant == "no_tail":
                    nc.sync.dma_start(cc_in[w * 128:(w + 1) * 128, :], aggc[:])
                    continue
                aggT = work.tile([128, KF, 128], dt.bfloat16, tag="aggT")
                for k in range(KF):
                    trp = ps_tr.tile([128, 128], dt.bfloat16, tag="tr")
                    nc.tensor.transpose(trp[:], aggc[:, k * 128:(k + 1) * 128],
                                        ident_bf[:])
                    nc.vector.tensor_copy(aggT[:, k, :], trp[:])
                h1 = ps_h.tile([128, H], dt.float32, tag="h")
                for k in range(KF):
                    nc.tensor.matmul(h1[:], lhsT=aggT[:, k, :], rhs=w1_sb[:, k, :],
                                     start=(k == 0), stop=(k == KF - 1))
                t1 = work.tile([128, H], dt.float32, tag="t1")
                nc.vector.scalar_tensor_tensor(t1[:], h1[:], ndst_t[:, w:w + 1],
                                               b1_bc[:], alu.mult, alu.add)
                yz = work.tile([128, H], dt.bfloat16, tag="yz")
                nc.scalar.activation(yz[:], t1[:],
                                     mybir.ActivationFunctionType.Relu,
                                     scale=nso_t[:, w:w + 1])
                yzT = work.tile([128, KH, 128], dt.bfloat16, tag="yzT")
                for k in range(KH):
                    trp2 = ps_tr.tile([128, 128], dt.bfloat16, tag="tr")
                    nc.tensor.transpose(trp2[:], yz[:, k * 128:(k + 1) * 128],
                                        ident_bf[:])
                    nc.vector.tensor_copy(yzT[:, k, :], trp2[:])
                zn = ps_h.tile([128, C], dt.float32, tag="zn")
                for k in range(KH):
                    nc.tensor.matmul(zn[:], lhsT=yzT[:, k, :], rhs=w2_sb[:, k, :],
                                     start=(k == 0), stop=(k == KH - 1))
                znb = work.tile([128, 256], dt.bfloat16, tag="znb")
                nc.vector.memset(znb[:], 0.0)
                nc.vector.tensor_copy(znb[:, :C], zn[:])
                nc.sync.dma_start(cc_in[w * 128:(w + 1) * 128, :], znb[:])

            # ---- halo exchange ----
            if variant == "no_cc" or num_devices == 1:
                nc.sync.dma_start(z_full[0:NT * 128, :], cc_in[:, :])
            else:
                nc.gpsimd.collective_compute(
                    "AllGather", alu.bypass,
                    replica_groups=[list(range(N_CORES))],
                    ins=[cc_in.opt()], outs=[z_full.opt()])

            # ---- layer 2 ----
            for w in range(NT):
                # z rows sit at 512B pitch in z_full; read only the first
                # 256B (the C real cols + pad) of each — elem_step > elem_size.
                xg2 = xg_pool.tile([128, B, 128], dt.bfloat16, tag="xg2")
                if variant == "no_l2gather_pure":
                    nc.vector.memset(xg2[:, 0, 0:16], 0.25)
                elif variant in ("no_gather", "no_l2gather"):
                    nc.vector.memset(xg2[:], 0.25)
                else:
                    for g in range(0, B, GC):
                        nb = min(GC, B - g)
                        c0 = (w * B + g) * 8
                        nc.gpsimd.dma_gather(
                            xg2[:, g:g + nb, :], z_full[:, 0:128],
                            idx2_t[:, c0:c0 + nb * 8],
                            nb * 128, nb * 128, 128, elem_step=256,
                            queue_num=(w * ((B + GC - 1) // GC) + g // GC) % 4)
                m2 = m_pool.tile([128, B, 128], dt.bfloat16, tag="m")
                nc.vector.tensor_tensor(
                    m2[:], iota_bf[:],
                    dlbf_t[:, w * B:(w + 1) * B].broadcast_to((128, B, 128)),
                    alu.is_equal)
                agg2 = ps_agg.tile([128, C], dt.float32, tag="agg")
                for b in range(B):
                    nc.tensor.matmul(agg2[:], lhsT=m2[:, b, :],
                                     rhs=xg2[:, b, 0:C],
                                     start=(b == 0), stop=(b == B - 1))
                ot = work.tile([128, C], dt.float32, tag="ot")
                nc.vector.scalar_tensor_tensor(ot[:], agg2[:],
                                               ndst_t[:, w:w + 1], b2_bc[:],
                                               alu.mult, alu.add)
                nc.sync.dma_start(out_d.ap()[w * 128:(w + 1) * 128, :], ot[:])

    nc.compile()
    return nc


def kernel(features, W1, b1, W2, b2, src, dst, **_):
    import time
    from concourse.bass_utils import run_bass_kernel_spmd

    t0 = time.time()
    shapes, in_maps = _prep(features, W1, b1, W2, b2, src, dst)
    t1 = time.time()
    nc = _build(shapes["N"], shapes["F"], shapes["H"], shapes["C"],
                shapes["NT"], shapes["B"], shapes["NE"])
    t2 = time.time()
    res = run_bass_kernel_spmd(nc, in_maps, core_ids=list(range(N_CORES)))
    t3 = time.time()
    npc = shapes["npc"]
    out = np.concatenate([res.results[c]["out"][:npc] for c in range(N_CORES)], 0)
    LAST_STATS.update(prep_s=t1 - t0, build_s=t2 - t1, run_s=t3 - t2,
                      B=shapes["B"], NE=shapes["NE"])
    return np.ascontiguousarray(out.astype(np.float32))



# revision 1
# speedup vs baseline: 3.8844x; 3.8844x over previous
"""Two-layer GCN (DGL GraphConv norm='both') on 8 Trainium2 NeuronCores.

Strategy (graph/data parallel, per sharding hint):
  - Nodes are range-partitioned across the 8 cores (1250 each); each core
    owns the dst-side segment_sum for its node range.
  - Host pre-sorts each core's incoming edges by dst, groups them into
    128-row dst windows, and pads each window's edge list to a uniform
    block count so all 8 cores share one SPMD program.
  - Layer-1 aggregation: dma_gather pulls (norm_src-scaled) source feature
    rows (bf16) from HBM; a per-block one-hot matrix M (built on-chip with
    iota + is_equal from precomputed local-dst ids) turns the segment sum
    into TensorEngine matmuls accumulating in PSUM:
        agg[dst,:] += M[edge,dst]^T @ Xg[edge,:]
  - H1 = (agg @ W1) * norm_dst + b1 (norm commutes through W1), ReLU, then
    z = (relu * norm_src) @ W2 is computed locally and AllGathered (bf16,
    padded to 128 cols so gather rows are 256B-aligned).
  - Layer-2 aggregation: same gather + one-hot matmul scheme over z,
    then out = agg2 * norm_dst + b2.
"""

import numpy as np
import ml_dtypes

BF16 = ml_dtypes.bfloat16
N_CORES = 8

LAST_STATS = {}


def _prep(features, W1, b1, W2, b2, src, dst):
    N, F = features.shape
    H = W1.shape[1]
    C = W2.shape[1]
    E = src.shape[0]
    assert N % N_CORES == 0
    npc = N // N_CORES            # nodes per core
    NT = (npc + 127) // 128       # dst windows per core
    npc_pad = NT * 128

    src = np.asarray(src, np.int64)
    dst = np.asarray(dst, np.int64)

    out_deg = np.bincount(src, minlength=N).astype(np.float32)
    in_deg = np.bincount(dst, minlength=N).astype(np.float32)
    norm_src = (1.0 / np.sqrt(np.clip(out_deg, 1.0, None))).astype(np.float32)
    norm_dst = (1.0 / np.sqrt(np.clip(in_deg, 1.0, None))).astype(np.float32)

    # norm_src folded into the gathered feature table (host-side sharding prep)
    featb = (np.asarray(features, np.float32) * norm_src[:, None]).astype(BF16)

    order = np.argsort(dst, kind="stable")
    ds = dst[order]
    ss = src[order]

    # per (core, window) counts
    i0 = np.empty((N_CORES, NT), np.int64)
    i1 = np.empty((N_CORES, NT), np.int64)
    for c in range(N_CORES):
        for w in range(NT):
            lo = c * npc + w * 128
            hi = min(lo + 128, (c + 1) * npc)
            i0[c, w] = np.searchsorted(ds, lo, "left")
            i1[c, w] = np.searchsorted(ds, hi, "left")
    counts = i1 - i0
    B = int(np.ceil(counts.max() / 128))      # blocks per window (uniform)
    EW = B * 128                              # padded edges per window
    NE = NT * EW                              # padded edges per core
    NBLK = NT * B

    idx1 = np.zeros((N_CORES, 128, NE // 16), np.int16)
    idx2 = np.zeros((N_CORES, 128, NE // 16), np.int16)
    dlw = np.zeros((N_CORES, 128, NBLK), BF16)
    ndst_t = np.zeros((N_CORES, 128, NT), np.float32)
    nso_t = np.zeros((N_CORES, 128, NT), np.float32)

    for c in range(N_CORES):
        s1 = np.zeros(NE, np.int64)
        dl = np.full(NE, -1.0, np.float32)
        for w in range(NT):
            a, b = i0[c, w], i1[c, w]
            cnt = b - a
            pos = w * EW
            s1[pos:pos + cnt] = ss[a:b]
            dl[pos:pos + cnt] = (ds[a:b] - (c * npc + w * 128)).astype(np.float32)
        s2 = (s1 // npc) * npc_pad + (s1 % npc)   # row in AllGathered z
        idx1[c] = np.tile(s1.reshape(NE // 16, 16).T.astype(np.int16), (8, 1))
        idx2[c] = np.tile(s2.reshape(NE // 16, 16).T.astype(np.int16), (8, 1))
        dlw[c] = dl.reshape(NBLK, 128).T.astype(BF16)

        own_nd = np.ones(npc_pad, np.float32)
        own_nd[:npc] = norm_dst[c * npc:(c + 1) * npc]
        own_ns = np.ones(npc_pad, np.float32)
        own_ns[:npc] = norm_src[c * npc:(c + 1) * npc]
        ndst_t[c] = own_nd.reshape(NT, 128).T
        nso_t[c] = own_ns.reshape(NT, 128).T

    shapes = dict(N=N, F=F, H=H, C=C, NT=NT, B=B, NE=NE, npc=npc)
    in_maps = []
    W1b = np.asarray(W1, np.float32).astype(BF16)
    W2b = np.asarray(W2, np.float32).astype(BF16)
    b1f = np.asarray(b1, np.float32)[None, :]
    b2f = np.asarray(b2, np.float32)[None, :]
    for c in range(N_CORES):
        in_maps.append(dict(
            featb=featb, idx1=idx1[c], idx2=idx2[c], dlbf=dlw[c],
            ndst=ndst_t[c], nso=nso_t[c],
            w1=W1b, w2=W2b, b1=b1f, b2=b2f,
        ))
    return shapes, in_maps


def _build(N, F, H, C, NT, B, NE, variant="full", num_devices=N_CORES):
    import concourse.bacc as bacc
    import concourse.mybir as mybir
    from concourse import tile

    dt = mybir.dt
    alu = mybir.AluOpType
    KF = F // 128   # feat chunks
    KH = H // 128   # hidden chunks
    EW = B * 128
    ZROWS = N_CORES * NT * 128

    nc = bacc.Bacc("TRN2", target_bir_lowering=False, debug=False,
                   num_devices=num_devices, num_swdge_queues=4)

    featb_d = nc.dram_tensor("featb", [N, F], dt.bfloat16, kind="ExternalInput")
    idx1_d = nc.dram_tensor("idx1", [128, NE // 16], dt.int16, kind="ExternalInput")
    idx2_d = nc.dram_tensor("idx2", [128, NE // 16], dt.int16, kind="ExternalInput")
    dlbf_d = nc.dram_tensor("dlbf", [128, NT * B], dt.bfloat16, kind="ExternalInput")
    ndst_d = nc.dram_tensor("ndst", [128, NT], dt.float32, kind="ExternalInput")
    nso_d = nc.dram_tensor("nso", [128, NT], dt.float32, kind="ExternalInput")
    w1_d = nc.dram_tensor("w1", [F, H], dt.bfloat16, kind="ExternalInput")
    w2_d = nc.dram_tensor("w2", [H, C], dt.bfloat16, kind="ExternalInput")
    b1_d = nc.dram_tensor("b1", [1, H], dt.float32, kind="ExternalInput")
    b2_d = nc.dram_tensor("b2", [1, C], dt.float32, kind="ExternalInput")
    out_d = nc.dram_tensor("out", [NT * 128, C], dt.float32, kind="ExternalOutput")

    with tile.TileContext(nc) as tc:
        with (
            tc.tile_pool(name="const", bufs=1) as const,
            tc.tile_pool(name="dram", bufs=1, space="DRAM") as dram,
            tc.tile_pool(name="xg", bufs=3) as xg_pool,
            tc.tile_pool(name="mp", bufs=3) as m_pool,
            tc.tile_pool(name="work", bufs=3) as work,
            tc.tile_pool(name="ps_agg", bufs=2, space="PSUM") as ps_agg,
            tc.tile_pool(name="ps_tr", bufs=2, space="PSUM") as ps_tr,
            tc.tile_pool(name="ps_h", bufs=2, space="PSUM") as ps_h,
        ):
            # ---- constants ----
            idx1_t = const.tile([128, NE // 16], dt.int16)
            nc.sync.dma_start(idx1_t[:], idx1_d.ap())
            idx2_t = const.tile([128, NE // 16], dt.int16)
            nc.sync.dma_start(idx2_t[:], idx2_d.ap())
            dlbf_t = const.tile([128, NT * B], dt.bfloat16)
            nc.sync.dma_start(dlbf_t[:], dlbf_d.ap())
            ndst_t = const.tile([128, NT], dt.float32)
            nc.sync.dma_start(ndst_t[:], ndst_d.ap())
            nso_t = const.tile([128, NT], dt.float32)
            nc.sync.dma_start(nso_t[:], nso_d.ap())

            w1_sb = const.tile([128, KF, H], dt.bfloat16)
            nc.sync.dma_start(w1_sb[:], w1_d.ap().rearrange("(k p) n -> p k n", p=128))
            w2_sb = const.tile([128, KH, C], dt.bfloat16)
            nc.sync.dma_start(w2_sb[:], w2_d.ap().rearrange("(k p) n -> p k n", p=128))
            b1_sb = const.tile([1, H], dt.float32)
            nc.sync.dma_start(b1_sb[:], b1_d.ap())
            b2_sb = const.tile([1, C], dt.float32)
            nc.sync.dma_start(b2_sb[:], b2_d.ap())

            iota_bf = const.tile([128, B, 128], dt.bfloat16)
            nc.gpsimd.iota(iota_bf[:], pattern=[[0, B], [1, 128]], base=0,
                           channel_multiplier=0,
                           allow_small_or_imprecise_dtypes=True)
            iota_col = const.tile([128, 1], dt.float32)
            nc.gpsimd.iota(iota_col[:], pattern=[[0, 1]], base=0,
                           channel_multiplier=1,
                           allow_small_or_imprecise_dtypes=True)
            ident_bf = const.tile([128, 128], dt.bfloat16)
            nc.vector.tensor_scalar(ident_bf[:], iota_bf[:, 0, :], iota_col[:],
                                    None, alu.is_equal)

            # bias rows broadcast across partitions via ones-column matmul
            ones_sb = const.tile([1, 128], dt.float32)
            nc.vector.memset(ones_sb[:], 1.0)
            b1_ps = ps_h.tile([128, H], dt.float32, tag="h")
            nc.tensor.matmul(b1_ps[:], lhsT=ones_sb[:], rhs=b1_sb[:],
                             start=True, stop=True)
            b1_bc = const.tile([128, H], dt.float32)
            nc.vector.tensor_copy(b1_bc[:], b1_ps[:])
            b2_ps = ps_h.tile([128, C], dt.float32, tag="zn")
            nc.tensor.matmul(b2_ps[:], lhsT=ones_sb[:], rhs=b2_sb[:],
                             start=True, stop=True)
            b2_bc = const.tile([128, C], dt.float32)
            nc.vector.tensor_copy(b2_bc[:], b2_ps[:])

            # z rows padded to 256 bf16 cols = 512B: dma_gather rows below
            # 512B crash the Q7 ucode; gathered cols C.. are never consumed.
            cc_in = dram.tile([NT * 128, 256], dt.bfloat16)
            z_full = dram.tile([ZROWS, 256], dt.bfloat16, addr_space="Shared")

            # ---- layer 1 ----
            # dma_gather is limited to 1024 idxs per instruction (64 descs per
            # SDMA lane = the single_packet ceiling); larger crashes the Q7.
            GC = 8  # blocks (of 128 idxs) per gather instruction

            for w in range(NT):
                xg = xg_pool.tile([128, B, F], dt.bfloat16, tag="xg")
                if variant == "no_gather":
                    nc.vector.memset(xg[:], 0.25)
                elif variant == "no_l1gather":
                    nc.vector.memset(xg[:, 0, 0:16], 0.25)
                else:
                    for g in range(0, B, GC):
                        nb = min(GC, B - g)
                        c0 = (w * B + g) * 8
                        nc.gpsimd.dma_gather(
                            xg[:, g:g + nb, :], featb_d.ap(),
                            idx1_t[:, c0:c0 + nb * 8],
                            nb * 128, nb * 128, F,
                            queue_num=(w * ((B + GC - 1) // GC) + g // GC) % 4)
                m1 = m_pool.tile([128, B, 128], dt.bfloat16, tag="m")
                nc.vector.tensor_tensor(
                    m1[:], iota_bf[:],
                    dlbf_t[:, w * B:(w + 1) * B].broadcast_to((128, B, 128)),
                    alu.is_equal)
                agg = ps_agg.tile([128, F], dt.float32, tag="agg")
                NMM = 32 if variant == "tiny_mm" else F
                for b in range(B):
                    nc.tensor.matmul(agg[:, 0:NMM], lhsT=m1[:, b, :],
                                     rhs=xg[:, b, 0:NMM],
                                     start=(b == 0), stop=(b == B - 1))
                aggc = work.tile([128, F], dt.bfloat16, tag="aggc")
                nc.vector.tensor_copy(aggc[:], agg[:])
                if variant == "no_tail":
                    nc.sync.dma_start(cc_in[w * 128:(w + 1) * 128, :], aggc[:])
                    continue
                aggT = work.tile([128, KF, 128], dt.bfloat16, tag="aggT")
                for k in range(KF):
                    trp = ps_tr.tile([128, 128], dt.bfloat16, tag="tr")
                    nc.tensor.transpose(trp[:], aggc[:, k * 128:(k + 1) * 128],
                                        ident_bf[:])
                    nc.vector.tensor_copy(aggT[:, k, :], trp[:])
                h1 = ps_h.tile([128, H], dt.float32, tag="h")
                for k in range(KF):
                    nc.tensor.matmul(h1[:], lhsT=aggT[:, k, :], rhs=w1_sb[:, k, :],
                                     start=(k == 0), stop=(k == KF - 1))
                t1 = work.tile([128, H], dt.float32, tag="t1")
                nc.vector.scalar_tensor_tensor(t1[:], h1[:], ndst_t[:, w:w + 1],
                                               b1_bc[:], alu.mult, alu.add)
                yz = work.tile([128, H], dt.bfloat16, tag="yz")
                nc.scalar.activation(yz[:], t1[:],
                                     mybir.ActivationFunctionType.Relu,
                                     scale=nso_t[:, w:w + 1])
                yzT = work.tile([128, KH, 128], dt.bfloat16, tag="yzT")
                for k in range(KH):
                    trp2 = ps_tr.tile([128, 128], dt.bfloat16, tag="tr")
                    nc.tensor.transpose(trp2[:], yz[:, k * 128:(k + 1) * 128],
                                        ident_bf[:])
                    nc.vector.tensor_copy(yzT[:, k, :], trp2[:])
                zn = ps_h.tile([128, C], dt.float32, tag="zn")
                for k in range(KH):
                    nc.tensor.matmul(zn[:], lhsT=yzT[:, k, :], rhs=w2_sb[:, k, :],
                                     start=(k == 0), stop=(k == KH - 1))
                znb = work.tile([128, 256], dt.bfloat16, tag="znb")
                nc.vector.memset(znb[:], 0.0)
                nc.vector.tensor_copy(znb[:, :C], zn[:])
                nc.sync.dma_start(cc_in[w * 128:(w + 1) * 128, :], znb[:])

            # ---- halo exchange ----
            if variant == "no_cc" or num_devices == 1:
                nc.sync.dma_start(z_full[0:NT * 128, :], cc_in[:, :])
            else:
                nc.gpsimd.collective_compute(
                    "AllGather", alu.bypass,
                    replica_groups=[list(range(N_CORES))],
                    ins=[cc_in.opt()], outs=[z_full.opt()])

            # ---- layer 2 ----
            for w in range(NT):
                # z rows sit at 512B pitch in z_full; read only the first
                # 256B (the C real cols + pad) of each — elem_step > elem_size.
                xg2 = xg_pool.tile([128, B, 128], dt.bfloat16, tag="xg2")
                if variant == "no_l2gather_pure":
                    nc.vector.memset(xg2[:, 0, 0:16], 0.25)
                elif variant in ("no_gather", "no_l2gather"):
                    nc.vector.memset(xg2[:], 0.25)
                else:
                    for g in range(0, B, GC):
                        nb = min(GC, B - g)
                        c0 = (w * B + g) * 8
                        nc.gpsimd.dma_gather(
                            xg2[:, g:g + nb, :], z_full[:, 0:128],
                            idx2_t[:, c0:c0 + nb * 8],
                            nb * 128, nb * 128, 128, elem_step=256,
                            queue_num=(w * ((B + GC - 1) // GC) + g // GC) % 4)
                m2 = m_pool.tile([128, B, 128], dt.bfloat16, tag="m")
                nc.vector.tensor_tensor(
                    m2[:], iota_bf[:],
                    dlbf_t[:, w * B:(w + 1) * B].broadcast_to((128, B, 128)),
                    alu.is_equal)
                agg2 = ps_agg.tile([128, C], dt.float32, tag="agg")
                for b in range(B):
                    nc.tensor.matmul(agg2[:], lhsT=m2[:, b, :],
                                     rhs=xg2[:, b, 0:C],
                                     start=(b == 0), stop=(b == B - 1))
                ot = work.tile([128, C], dt.float32, tag="ot")
                nc.vector.scalar_tensor_tensor(ot[:], agg2[:],
                                               ndst_t[:, w:w + 1], b2_bc[:],
                                               alu.mult, alu.add)
                nc.sync.dma_start(out_d.ap()[w * 128:(w + 1) * 128, :], ot[:])

    nc.compile()
    return nc


def kernel(features, W1, b1, W2, b2, src, dst, **_):
    import time
    from concourse.bass_utils import run_bass_kernel_spmd

    t0 = time.time()
    shapes, in_maps = _prep(features, W1, b1, W2, b2, src, dst)
    t1 = time.time()
    nc = _build(shapes["N"], shapes["F"], shapes["H"], shapes["C"],
                shapes["NT"], shapes["B"], shapes["NE"])
    t2 = time.time()
    res = run_bass_kernel_spmd(nc, in_maps, core_ids=list(range(N_CORES)))
    t3 = time.time()
    npc = shapes["npc"]
    out = np.concatenate([res.results[c]["out"][:npc] for c in range(N_CORES)], 0)
    LAST_STATS.update(prep_s=t1 - t0, build_s=t2 - t1, run_s=t3 - t2,
                      B=shapes["B"], NE=shapes["NE"])
    return np.ascontiguousarray(out.astype(np.float32))

